# revision 4
# baseline (speedup 1.0000x reference)
"""GroupedExperts MoE kernel for Trainium2 (8 NeuronCores, expert-parallel).

Reference computation (per expert e):
    h   = x[e] @ W1[e] + b1[e]              # [T, 2D]
    glu = min(h[..., ::2], 7)
    lin = clip(h[..., 1::2], -7, 7)
    s   = glu * sigmoid(1.702 * glu) * (lin + 1)
    out = s @ W2[e] + b2[e]                 # [T, D]

Shapes: E=8, T=2048, D=2048.  One expert per NeuronCore, no cross-core comm.

Device dataflow is fully transposed (features on partitions, tokens on the
free dim) so no on-chip transposes are needed:
    MM1:  hT[f_chunk] = sum_k W1[k, f_chunk].T @ xT[k]     (W1 stationary)
    MM2:  outT[d_chunk] = sum_f W2[f, d_chunk].T @ sT[f]   (W2 stationary)
W1 is de-interleaved on the host into glu/lin halves, so SwiGLU becomes
elementwise between two separate PSUM tiles.

All matmul operands are bf16 (same 1 row/cycle PE rate as float32r, but
half the HBM traffic and SBUF footprint), with fp32 PSUM accumulation.
The whole T=2048 token range is processed in a single pass, so W1/W2 are
loaded exactly once (48MB total vs 128MB for a two-pass fp32 schedule).
MM2 weight panels are prefetched two iterations ahead so the in-order
sync DMA ring never starves the PE at a d-chunk boundary.
"""

import os
import sys

import numpy as np

for _p in ("/opt/trn_rl_repo", "/root/.axon_site/_ro/trn_rl_repo"):
    if os.path.isdir(_p) and _p not in sys.path:
        sys.path.append(_p)

import ml_dtypes  # noqa: E402

import concourse.bass as bass  # noqa: E402
import concourse.mybir as mybir  # noqa: E402
import concourse.tile as tile  # noqa: E402
from concourse import bacc  # noqa: E402
from concourse.bass_utils import run_bass_kernel_spmd  # noqa: E402

E = 8
T = 2048
D = 2048
P = 128
KO = D // P      # 16 k-chunks (contraction over D)
FO = D // P      # 16 feature chunks per glu/lin half
DO = D // P      # 16 output-feature chunks
NS4 = T // 512   # 4 psum-width token sub-tiles across the full T

ALPHA = 1.702
LIMIT = 7.0

TRACE = False          # test.py sets True to capture an NTFF profile
LAST_RESULTS = None    # test.py reads exec_time_ns / trace path from here

_CACHE = {}

f32 = mybir.dt.float32
bf16 = mybir.dt.bfloat16
BF16_NP = ml_dtypes.bfloat16


def _emit(tc, xt, w1g, w1l, w2p, b1g, b1l, b2, outT):
    from contextlib import ExitStack

    ctx = ExitStack()
    nc = tc.nc
    Silu = mybir.ActivationFunctionType.Silu
    Ident = mybir.ActivationFunctionType.Identity
    add = mybir.AluOpType.add
    amin = mybir.AluOpType.min
    amax = mybir.AluOpType.max

    const_pool = ctx.enter_context(tc.tile_pool(name="const", bufs=1))
    x_pool = ctx.enter_context(tc.tile_pool(name="xp", bufs=KO))
    s_pool = ctx.enter_context(tc.tile_pool(name="sp", bufs=FO))
    w1_pool = ctx.enter_context(tc.tile_pool(name="w1p", bufs=8))
    w2_pool = ctx.enter_context(tc.tile_pool(name="w2pool", bufs=3))
    t_pool = ctx.enter_context(tc.tile_pool(name="tp", bufs=3))
    o_pool = ctx.enter_context(tc.tile_pool(name="op", bufs=4))
    ps_pool = ctx.enter_context(tc.tile_pool(name="ps", bufs=8, space="PSUM"))

    b1g_sb = const_pool.tile([P, FO], f32, name="b1g_sb")
    b1l_sb = const_pool.tile([P, FO], f32, name="b1l_sb")
    b2_sb = const_pool.tile([P, DO], f32, name="b2_sb")

    xts = [
        x_pool.tile([P, T], bf16, tag="xt", name=f"xt_{k}") for k in range(KO)
    ]
    s_tiles = [
        s_pool.tile([P, T], bf16, tag="s", name=f"s_{f}") for f in range(FO)
    ]

    # ---- head DMA schedule.  Issue latency is the binding constraint at
    # the head (~0.6us of sequencer time per dma_start), so the three
    # input streams go out on three independent engine queues in
    # parallel: W1 panels on sync, the x panel on vector (idle until the
    # first swiglu ~40us in), biases on scalar.  xt[0] is split into four
    # 512-col chunks so the very first matmul only waits for 128KB of x.
    w1t = {}
    wg0 = w1_pool.tile([P, KO, P], bf16, tag="w1", name="wg_0")
    wl0 = w1_pool.tile([P, KO, P], bf16, tag="w1", name="wl_0")
    w1t[0] = (wg0, wl0)
    nc.sync.dma_start(wg0[:, 0:4, :], w1g[0, :, 0:4, :])
    nc.sync.dma_start(wl0[:, 0:4, :], w1l[0, :, 0:4, :])
    nc.sync.dma_start(wg0[:, 4:KO, :], w1g[0, :, 4:KO, :])
    nc.sync.dma_start(wl0[:, 4:KO, :], w1l[0, :, 4:KO, :])
    wg1 = w1_pool.tile([P, KO, P], bf16, tag="w1", name="wg_1")
    nc.sync.dma_start(wg1[:], w1g[1])
    wl1 = w1_pool.tile([P, KO, P], bf16, tag="w1", name="wl_1")
    nc.sync.dma_start(wl1[:], w1l[1])
    w1t[1] = (wg1, wl1)
    for ns in range(NS4):
        nc.scalar.dma_start(
            xts[0][:, ns * 512 : (ns + 1) * 512],
            xt[0, :, ns * 512 : (ns + 1) * 512],
        )
    for k in range(1, KO):
        nc.scalar.dma_start(xts[k][:], xt[k])
    nc.gpsimd.dma_start(b1g_sb[:], b1g)
    nc.gpsimd.dma_start(b1l_sb[:], b1l)
    nc.gpsimd.dma_start(b2_sb[:], b2)

    def swiglu_chunk(f, pgj, plj, scol):
        # tg = min(hg + b1g, LIMIT)
        tg = t_pool.tile([P, 512], f32, tag="tg", name=f"tg_{f}_{scol}")
        nc.vector.tensor_scalar(
            out=tg[:],
            in0=pgj[:],
            scalar1=b1g_sb[:, f : f + 1],
            scalar2=LIMIT,
            op0=add,
            op1=amin,
        )
        # yg = silu(ALPHA*tg) = ALPHA * tg * sigmoid(ALPHA*tg)
        yg = t_pool.tile([P, 512], f32, tag="yg", name=f"yg_{f}_{scol}")
        nc.scalar.activation(out=yg[:], in_=tg[:], func=Silu, scale=ALPHA)
        # tl = (clip(hl, -7, 7) + 1)/ALPHA
        #    = clip((hl + b1l + 1)/ALPHA, (-7+1)/ALPHA, (7+1)/ALPHA)
        tl = t_pool.tile([P, 512], f32, tag="tl", name=f"tl_{f}_{scol}")
        nc.scalar.activation(
            out=tl[:],
            in_=plj[:],
            func=Ident,
            bias=b1l_sb[:, f : f + 1],
            scale=1.0 / ALPHA,
        )
        nc.vector.tensor_scalar(
            out=tl[:],
            in0=tl[:],
            scalar1=(LIMIT + 1.0) / ALPHA,
            scalar2=(-LIMIT + 1.0) / ALPHA,
            op0=amin,
            op1=amax,
        )
        # s = yg * tl  (the ALPHA factors cancel), cast to bf16 for MM2
        nc.vector.tensor_mul(
            out=s_tiles[f][:, scol : scol + 512], in0=yg[:], in1=tl[:]
        )

    # ---- MM1 f=0: k-outer over the full token width (8 PSUM banks) so
    # each arriving xt[k] panel unlocks 8 matmuls and the PE streams
    # behind the DMA with no double-wait.
    pg = [
        ps_pool.tile([P, 512], f32, tag="ps", name=f"pg0_{ns}")
        for ns in range(NS4)
    ]
    pl = [
        ps_pool.tile([P, 512], f32, tag="ps", name=f"pl0_{ns}")
        for ns in range(NS4)
    ]
    for k in range(KO):
        for part, w in ((pg, wg0), (pl, wl0)):
            for ns in range(NS4):
                nc.tensor.matmul(
                    part[ns][:],
                    w[:, k, :],
                    xts[k][:, ns * 512 : (ns + 1) * 512],
                    start=(k == 0),
                    stop=(k == KO - 1),
                )
    for ns in range(NS4):
        swiglu_chunk(0, pg[ns], pl[ns], ns * 512)

    # ---- MM1 f=1..15: half-token jobs (4 PSUM banks each) so two jobs
    # double-buffer across the 8 banks and the PE never waits for the
    # swiglu drain.
    for f in range(1, FO):
        if f + 1 < FO:
            wgn = w1_pool.tile([P, KO, P], bf16, tag="w1", name=f"wg_{f+1}")
            nc.sync.dma_start(wgn[:], w1g[f + 1])
            wln = w1_pool.tile([P, KO, P], bf16, tag="w1", name=f"wl_{f+1}")
            nc.sync.dma_start(wln[:], w1l[f + 1])
            w1t[f + 1] = (wgn, wln)
        wg, wl = w1t.pop(f)
        for h in range(2):
            pg2 = [
                ps_pool.tile([P, 512], f32, tag="ps", name=f"pg_{f}_{h}_{ns}")
                for ns in range(2)
            ]
            pl2 = [
                ps_pool.tile([P, 512], f32, tag="ps", name=f"pl_{f}_{h}_{ns}")
                for ns in range(2)
            ]
            for k in range(KO):
                for part, w in ((pg2, wg), (pl2, wl)):
                    for ns in range(2):
                        c = h * 1024 + ns * 512
                        nc.tensor.matmul(
                            part[ns][:],
                            w[:, k, :],
                            xts[k][:, c : c + 512],
                            start=(k == 0),
                            stop=(k == KO - 1),
                        )
            for ns in range(2):
                swiglu_chunk(f, pg2[ns], pl2[ns], h * 1024 + ns * 512)

    # ---- MM2 + bias: outT[d] = sum_f W2[f, d].T @ sT[f] + b2[d]
    # Weight panels prefetched 2 iterations ahead; the sync ring order is
    # w2(d+2), out(d, q=0..3), so weight transfers are never queued behind
    # more than one iteration of output traffic.
    w2t = {}
    for dpre in range(2):
        w2t[dpre] = w2_pool.tile([P, FO, P], bf16, tag="w2", name=f"w2_{dpre}")
        nc.sync.dma_start(w2t[dpre][:], w2p[dpre])
    for d in range(DO):
        if d + 2 < DO:
            w2t[d + 2] = w2_pool.tile(
                [P, FO, P], bf16, tag="w2", name=f"w2_{d+2}"
            )
            nc.sync.dma_start(w2t[d + 2][:], w2p[d + 2])
        for q in range(NS4):
            po = ps_pool.tile([P, 512], f32, tag="ps", name=f"po_{d}_{q}")
            for f in range(FO):
                nc.tensor.matmul(
                    po[:],
                    w2t[d][:, f, :],
                    s_tiles[f][:, q * 512 : (q + 1) * 512],
                    start=(f == 0),
                    stop=(f == FO - 1),
                )
            ot = o_pool.tile([P, 512], f32, tag="o", name=f"ot_{d}_{q}")
            nc.scalar.activation(
                out=ot[:], in_=po[:], func=Ident, bias=b2_sb[:, d : d + 1]
            )
            nc.sync.dma_start(
                outT[d, :, q * 512 : (q + 1) * 512], ot[:]
            )
        del w2t[d]

    ctx.close()


def _build():
    if "nc" in _CACHE:
        return _CACHE["nc"]
    nc = bacc.Bacc(
        "TRN2",
        target_bir_lowering=False,
        debug=False,
        enable_asserts=False,
        num_devices=E,
    )
    xt = nc.dram_tensor("xt", (KO, P, T), bf16, kind="ExternalInput").ap()
    w1g = nc.dram_tensor("w1g", (FO, P, KO, P), bf16, kind="ExternalInput").ap()
    w1l = nc.dram_tensor("w1l", (FO, P, KO, P), bf16, kind="ExternalInput").ap()
    w2p = nc.dram_tensor("w2p", (DO, P, FO, P), bf16, kind="ExternalInput").ap()
    b1g = nc.dram_tensor("b1g", (P, FO), f32, kind="ExternalInput").ap()
    b1l = nc.dram_tensor("b1l", (P, FO), f32, kind="ExternalInput").ap()
    b2 = nc.dram_tensor("b2", (P, DO), f32, kind="ExternalInput").ap()
    outT = nc.dram_tensor("outT", (DO, P, T), f32, kind="ExternalOutput").ap()
    with tile.TileContext(nc) as tc:
        _emit(tc, xt, w1g, w1l, w2p, b1g, b1l, b2, outT)
    nc.compile()
    _CACHE["nc"] = nc
    return nc


def _pack_w(w):
    # [K, F] -> [fo, p, ko, m] with K = ko*128 + p, F = fo*128 + m
    return np.ascontiguousarray(
        w.reshape(KO, P, FO, P).transpose(2, 1, 0, 3).astype(BF16_NP)
    )


def _pack_b(b):
    # [F] -> [p, fo]
    return np.ascontiguousarray(b.reshape(FO, P).T.astype(np.float32))


def kernel(x, mlp1_weight, mlp1_bias, mlp2_weight, mlp2_bias):
    global LAST_RESULTS
    x = np.asarray(x, np.float32)
    mlp1_weight = np.asarray(mlp1_weight, np.float32)
    mlp1_bias = np.asarray(mlp1_bias, np.float32)
    mlp2_weight = np.asarray(mlp2_weight, np.float32)
    mlp2_bias = np.asarray(mlp2_bias, np.float32)

    nc = _build()
    in_maps = []
    for e in range(E):
        w1 = mlp1_weight[e].reshape(D, 2 * D // 2, 2)  # [K, F, 2] even/odd
        b1 = mlp1_bias[e].reshape(D, 2)
        in_maps.append(
            {
                "xt": np.ascontiguousarray(x[e].T.astype(BF16_NP)).reshape(
                    KO, P, T
                ),
                "w1g": _pack_w(np.ascontiguousarray(w1[:, :, 0])),
                "w1l": _pack_w(np.ascontiguousarray(w1[:, :, 1])),
                "w2p": _pack_w(mlp2_weight[e]),
                "b1g": _pack_b(np.ascontiguousarray(b1[:, 0])),
                "b1l": _pack_b((np.ascontiguousarray(b1[:, 1]) + 1.0) / ALPHA),
                "b2": _pack_b(mlp2_bias[e]),
            }
        )

    res = run_bass_kernel_spmd(
        nc, in_maps, core_ids=list(range(E)), trace=TRACE
    )
    LAST_RESULTS = res
    out = np.stack(
        [res.results[e]["outT"].reshape(D, T).T for e in range(E)]
    )
    return np.ascontiguousarray(out)
